# revision 1
# baseline (speedup 1.0000x reference)
"""Trainium2 Bass kernel for nn_Cross_Attention_Network.

Reference computation (per batch element b of N=8, one per NeuronCore):
  An = relu(BN(A)) ; Bn = relu(BN(B))          # BN stats over (N, L) -> AllReduce
  q = An.T @ theta_w + theta_b                 # [L, IC]
  k = Bn.T @ phi_w + phi_b
  v = An.T @ g_w + g_b
  attn = softmax(q @ k.T, axis=-1)
  out = An.T + (attn @ v) @ W_w + W_b          # [L, C] -> emitted as [C, L]

Layout strategy (transpose-free):
  - A, B live in native [C, L]; BN+ReLU is one fused ACT op per [128, 2048] tile
    (Relu(x * scale_c + shift_c) with per-partition scale/bias).
  - qT, kT computed in [IC, L]: matmul(lhsT=theta_w[c,ic], rhs=An[c,l]).
  - scores computed directly transposed: S^T[m, l] = contraction over ic.
    Softmax runs max-free (|scores| <= ~12, exp is safe in fp32).
  - v is computed in [L, IC] with a literal ones-column appended; the y-stage
    matmuls then produce both y^T[ic, l] and the softmax denominator
    colsum[1, l] on the PE for free.
  - Final projection W_w gives out^T[c, l] == the required output layout;
    residual + bias fused into one DVE op per tile (An kept in fp32, so the
    dominant residual term carries no rounding).
Matmul operands are bf16 (full PE rate); accumulation/softmax scalars fp32.
"""

from contextlib import ExitStack

import ml_dtypes
import numpy as np

try:
    import concourse.bass as bass  # noqa: F401
except ImportError:  # pragma: no cover - fallback for bare containers
    import sys

    sys.path.insert(0, "/opt/trn_rl_repo")
    import concourse.bass as bass  # noqa: F401

import concourse.tile as tile
from concourse import bacc, mybir
from concourse.bass_utils import run_bass_kernel_spmd

N, C, L = 8, 512, 2048
IC = C // 2
P = 128
CO = C // P            # 4 channel tiles
ICO = IC // P          # 2 ic tiles
LC = 512               # l-chunk (PSUM bank width in fp32)
NLC = L // LC          # 4 l chunks
NM = L // P            # 16 m tiles
BN_EPS = 1e-5
F32 = mybir.dt.float32
BF16 = mybir.dt.bfloat16
WEIGHT_KEYS = ("theta_w", "phi_w", "g_w", "W_w")
DMA_SPLIT = 2
WORK_BUFS = 4


def emit_kernel(tc, outs, ins, n_cores, use_cc=True, sections=("pre", "attn", "close"), state=None):
    if state is None:
        state = {}
    if "pre" in sections:
        _emit_pre(tc, outs, ins, n_cores, use_cc, state)
    if "attn" in sections:
        _emit_attn(tc, outs, ins, n_cores, state)
    if "close" in sections:
        emit_close(state)
    return state


def emit_close(state):
    state["ctx"].close()


def _emit_pre(tc, outs, ins, n_cores, use_cc, st):
    """Emit the per-core program. `outs`/`ins` are dicts of DRAM APs."""
    nc = tc.nc
    ctx = ExitStack()
    a_dr = ins["A"].rearrange("(o p) l -> o p l", p=P)
    b_dr = ins["B"].rearrange("(o p) l -> o p l", p=P)
    out_dr = outs["out"].rearrange("(o p) l -> o p l", p=P)

    const = ctx.enter_context(tc.tile_pool(name="const", bufs=1))
    big = ctx.enter_context(tc.tile_pool(name="big", bufs=1))
    stat = ctx.enter_context(tc.tile_pool(name="stat", bufs=1))
    dram = ctx.enter_context(tc.tile_pool(name="dram", bufs=1, space="DRAM"))
    work = ctx.enter_context(tc.tile_pool(name="work", bufs=WORK_BUFS))
    psum_s = ctx.enter_context(tc.tile_pool(name="psum_s", bufs=2, space="PSUM"))
    psum_y = ctx.enter_context(tc.tile_pool(name="psum_y", bufs=1, space="PSUM"))
    psum_c = ctx.enter_context(tc.tile_pool(name="psum_c", bufs=1, space="PSUM"))
    psum_o = ctx.enter_context(tc.tile_pool(name="psum_o", bufs=3, space="PSUM"))
    raw_ctx = ExitStack()
    rawp = raw_ctx.enter_context(tc.tile_pool(name="raw", bufs=1))
    # araw persists (residual relu re-derives An fp32 from it at the out stage)
    araw = big.tile([P, CO, L], F32)

    # ---- persistent SBUF arrays ----
    anb = big.tile([P, CO, L], BF16)      # An bf16 (matmul streams)
    bnb = big.tile([P, CO, L], BF16)      # Bn bf16
    qT = big.tile([P, ICO, L], BF16)      # q^T [IC, L]
    kT = big.tile([P, ICO, L], BF16)      # k^T [IC, L]
    vv = big.tile([P, NM, IC + 1], BF16)  # v [L, IC | ones]
    thw = const.tile([P, CO, IC], BF16)
    phw = const.tile([P, CO, IC], BF16)
    gw = const.tile([P, CO, IC], BF16)
    ww = const.tile([P, ICO, C], BF16)
    thb = const.tile([P, ICO], F32)
    phb = const.tile([P, ICO], F32)
    wb = const.tile([P, CO], F32)
    gam = const.tile([P, CO], F32)
    bet = const.tile([P, CO], F32)
    epst = const.tile([P, 1], F32)

    braw = rawp.tile([P, CO, L], F32, tag="braw")

    # ---- input DMAs (A first: its stats collective is on the critical path) ----
    for o in range(CO):
        for h in range(DMA_SPLIT):
            hs = slice(h * (L // DMA_SPLIT), (h + 1) * (L // DMA_SPLIT))
            nc.sync.dma_start(out=araw[:, o, hs], in_=a_dr[o][:, hs])
    nc.sync.dma_start(out=thw, in_=ins["theta_w"].rearrange("(o p) i -> p o i", p=P))
    nc.sync.dma_start(out=gw, in_=ins["g_w"].rearrange("(o p) i -> p o i", p=P))
    nc.sync.dma_start(out=phw, in_=ins["phi_w"].rearrange("(o p) i -> p o i", p=P))
    nc.sync.dma_start(out=ww, in_=ins["W_w"].rearrange("(o p) c -> p o c", p=P))
    nc.sync.dma_start(out=thb, in_=ins["theta_b"].rearrange("(o p) -> p o", p=P))
    nc.sync.dma_start(out=phb, in_=ins["phi_b"].rearrange("(o p) -> p o", p=P))
    nc.sync.dma_start(out=wb, in_=ins["W_b"].rearrange("(o p) -> p o", p=P))
    nc.sync.dma_start(out=gam, in_=ins["bn_gamma"].rearrange("(o p) -> p o", p=P))
    nc.sync.dma_start(out=bet, in_=ins["bn_beta"].rearrange("(o p) -> p o", p=P))
    for o in range(CO):
        for h in range(DMA_SPLIT):
            hs = slice(h * (L // DMA_SPLIT), (h + 1) * (L // DMA_SPLIT))
            nc.sync.dma_start(out=braw[:, o, hs], in_=b_dr[o][:, hs])

    nc.vector.memset(epst, BN_EPS)
    nc.vector.memset(vv[:, :, IC : IC + 1], 1.0)
    wscr = stat.tile([P, LC], BF16, tag="warm_mm")
    wscr2 = stat.tile([P, LC], BF16, tag="warm_mm2")
    nc.vector.memset(wscr, 0.0)
    nc.vector.memset(wscr2, 0.0)
    for _ in range(12):
        wp = psum_o.tile([P, LC], F32, tag="op")
        nc.tensor.matmul(out=wp, lhsT=wscr[:, 0:P], rhs=wscr)
    # preload the Sqrt ACT table-set while input DMAs stream (the load is
    # ~2.7us and would otherwise land on the stats critical path)
    warm = stat.tile([P, 1], F32)
    nc.scalar.activation(out=warm, in_=epst, func=mybir.ActivationFunctionType.Sqrt)

    # ---- BN statistics: per-channel sum / sumsq over local L, AllReduce over cores
    NSUB = L // 512
    replica = [list(range(n_cores))]
    scale = {}
    shift = {}

    ccgs = {}

    def emit_stats_collect(key, src_t, cc_engine):
        ccin = stat.tile([P, CO, 2], F32, tag=f"ccin_{key}")
        for o in range(CO):
            st = work.tile([P, NSUB, 6], F32, tag="bnstats")
            for s in range(NSUB):
                nc.vector.bn_stats(out=st[:, s, :], in_=src_t[:, o, 512 * s : 512 * (s + 1)])
            mv = work.tile([P, 2], F32, tag="bnmv")
            nc.vector.bn_aggr(out=mv, in_=st)
            # sum = mean * L ; sumsq = (mean^2 + var) * L
            nc.vector.tensor_scalar_mul(out=ccin[:, o, 0:1], in0=mv[:, 0:1], scalar1=float(L))
            t2 = work.tile([P, 1], F32, tag="bnt2")
            nc.vector.scalar_tensor_tensor(
                out=t2, in0=mv[:, 0:1], scalar=mv[:, 0:1], in1=mv[:, 1:2],
                op0=mybir.AluOpType.mult, op1=mybir.AluOpType.add,
            )
            nc.vector.tensor_scalar_mul(out=ccin[:, o, 1:2], in0=t2, scalar1=float(L))
        ccg = stat.tile([P, CO, 2], F32, tag=f"ccg_{key}")
        if n_cores > 1 and use_cc:
            cc_i = dram.tile([P, CO * 2], F32, tag=f"cci_{key}")
            cc_o = dram.tile([P, CO * 2], F32, tag=f"cco_{key}")
            nc.sync.dma_start(out=cc_i, in_=ccin)
            cc_engine.collective_compute(
                "AllReduce", mybir.AluOpType.add, replica_groups=replica,
                ins=[cc_i.opt()], outs=[cc_o.opt()],
            )
            nc.sync.dma_start(out=ccg, in_=cc_o)
        else:
            nc.vector.tensor_copy(out=ccg, in_=ccin)
        ccgs[key] = ccg

    def emit_stats_finish(key):
        ccg = ccgs[key]
        inv_cnt = 1.0 / float(n_cores * L)
        mean = stat.tile([P, CO], F32, tag=f"mean_{key}")
        msq = stat.tile([P, CO], F32, tag=f"msq_{key}")
        var = stat.tile([P, CO], F32, tag=f"var_{key}")
        sc = stat.tile([P, CO], F32, tag=f"sc_{key}")
        sh = stat.tile([P, CO], F32, tag=f"sh_{key}")
        nc.vector.tensor_scalar_mul(out=mean, in0=ccg[:, :, 0], scalar1=inv_cnt)
        nc.vector.tensor_scalar_mul(out=msq, in0=ccg[:, :, 1], scalar1=inv_cnt)
        nc.vector.tensor_mul(out=var, in0=mean, in1=mean)
        nc.vector.tensor_sub(out=var, in0=msq, in1=var)
        # std = sqrt(var + eps) on ACT, rstd via accurate DVE reciprocal
        nc.scalar.activation(out=var, in_=var, func=mybir.ActivationFunctionType.Sqrt, bias=epst)
        nc.vector.reciprocal(out=var, in_=var)
        nc.vector.tensor_mul(out=sc, in0=gam, in1=var)
        nc.vector.tensor_mul(out=sh, in0=mean, in1=sc)
        nc.vector.tensor_sub(out=sh, in0=bet, in1=sh)
        scale[key], shift[key] = sc, sh

    def project_T(dst, w, bias_pp, src_t):
        # dst[ic, l] = sum_c w[c, ic] * src_t[c, l]; bias per-partition (ic)
        for t in range(ICO):
            for lc in range(NLC):
                ps = psum_o.tile([P, LC], F32, tag="op")
                for o in range(CO):
                    nc.tensor.matmul(
                        out=ps, lhsT=w[:, o, P * t : P * (t + 1)],
                        rhs=src_t[:, o, LC * lc : LC * (lc + 1)],
                        start=(o == 0), stop=(o == CO - 1),
                    )
                nc.scalar.activation(
                    out=dst[:, t, LC * lc : LC * (lc + 1)], in_=ps,
                    func=mybir.ActivationFunctionType.Identity,
                    bias=bias_pp[:, t : t + 1],
                )

    # Collect stats for BOTH sides and launch both AllReduces up front
    # (B's on the sync queue so it doesn't serialize behind A's on gpsimd);
    # consume the results later, when each side's relu actually needs them.
    emit_stats_collect("a", araw, nc.gpsimd)
    emit_stats_collect("b", braw, nc.gpsimd)
    emit_stats_finish("a")
    # HAM re-warm bridge: gate junk matmuls on the AR-A result so the PE is
    # un-throttled when the first projection matmuls issue
    nc.vector.tensor_scalar_mul(out=wscr2[:, 0 : CO * 2],
                                in0=ccgs["a"].rearrange("p a b -> p (a b)"),
                                scalar1=0.0)
    for _ in range(12):
        wp = psum_o.tile([P, LC], F32, tag="op")
        nc.tensor.matmul(out=wp, lhsT=wscr2[:, 0:P], rhs=wscr2)
    # A relu fused on ACT (DVE must stay free for the B-side bn_stats that
    # land during the qT phase)
    for o in range(CO):
        nc.scalar.activation(
            out=anb[:, o, :], in_=araw[:, o, :],
            func=mybir.ActivationFunctionType.Relu,
            scale=scale["a"][:, o : o + 1], bias=shift["a"][:, o : o + 1],
        )

    project_T(qT, thw, thb, anb)

    emit_stats_finish("b")
    nc.scalar.activation(out=warm, in_=epst, func=mybir.ActivationFunctionType.Exp)

    # v[m, ic] = sum_c An[c, m] g_w[c, ic]   (g_b folded into W_b host-side)
    # fills the PE while DVE chews the B stats
    for m in range(NM):
        ps = psum_o.tile([P, IC], F32, tag="op")
        for o in range(CO):
            nc.tensor.matmul(
                out=ps, lhsT=anb[:, o, P * m : P * (m + 1)], rhs=gw[:, o, :],
                start=(o == 0), stop=(o == CO - 1),
            )
        nc.scalar.copy(out=vv[:, m, 0:IC], in_=ps)

    # B relu on DVE in l-chunks (ACT is busy with v epilogues here)
    for o in range(CO):
        for lc in range(NLC):
            sl = slice(LC * lc, LC * (lc + 1))
            nc.vector.tensor_scalar(
                out=bnb[:, o, sl], in0=braw[:, o, sl],
                scalar1=scale["b"][:, o : o + 1], scalar2=shift["b"][:, o : o + 1],
                op0=mybir.AluOpType.mult, op1=mybir.AluOpType.add,
            )
            nc.vector.tensor_scalar_max(out=bnb[:, o, sl], in0=bnb[:, o, sl], scalar1=0.0)
    raw_ctx.close()

    project_T(kT, phw, phb, bnb)

    st.update(dict(ctx=ctx, work=work, psum_s=psum_s, psum_y=psum_y,
                   psum_c=psum_c, psum_o=psum_o, araw=araw, qT=qT, kT=kT,
                   vv=vv, ww=ww, wb=wb, scale=scale, shift=shift,
                   out_dr=out_dr))


def _emit_attn(tc, outs, ins, n_cores, st):
    nc = tc.nc
    (work, psum_s, psum_y, psum_c, psum_o, araw, qT, kT, vv, ww, wb,
     scale, shift, out_dr) = (
        st["work"], st["psum_s"], st["psum_y"], st["psum_c"], st["psum_o"],
        st["araw"], st["qT"], st["kT"], st["vv"], st["ww"], st["wb"],
        st["scale"], st["shift"], st["out_dr"])

    # ---- attention + output, per l-chunk ----
    for lc in range(NLC):
        lsl = slice(LC * lc, LC * (lc + 1))
        yp = psum_y.tile([P, ICO, LC], F32, tag="yp")
        cs = psum_c.tile([1, LC], F32, tag="cs")

        def s_mm(m):
            sp = psum_s.tile([P, LC], F32, tag="sp")
            for t in range(ICO):
                nc.tensor.matmul(
                    out=sp, lhsT=kT[:, t, P * m : P * (m + 1)], rhs=qT[:, t, lsl],
                    start=(t == 0), stop=(t == ICO - 1),
                )
            return sp

        # software pipeline: S[m+1] is emitted before y[m] so the PE streams
        # score matmuls while ACT computes exp[m]
        sp = s_mm(0)
        for m in range(NM):
            pt = work.tile([P, LC], BF16, tag="pt")
            nc.scalar.activation(out=pt, in_=sp, func=mybir.ActivationFunctionType.Exp)
            if m + 1 < NM:
                sp = s_mm(m + 1)
            for t in range(ICO):
                nc.tensor.matmul(
                    out=yp[:, t, :], lhsT=vv[:, m, P * t : P * (t + 1)], rhs=pt,
                    start=(m == 0), stop=(m == NM - 1), skip_group_check=True,
                )
            nc.tensor.matmul(
                out=cs, lhsT=vv[:, m, IC : IC + 1], rhs=pt,
                start=(m == 0), stop=(m == NM - 1), skip_group_check=True,
            )
        rec = work.tile([1, LC], F32, tag="rec")
        rsc = work.tile([1, LC], F32, tag="rsc")
        nc.vector.reciprocal_approx_accurate(out=rec, in_=cs, scratch=rsc)
        rb = work.tile([P, LC], F32, tag="rb")
        nc.gpsimd.partition_broadcast(out_ap=rb, in_ap=rec)
        yt = work.tile([P, ICO, LC], BF16, tag="ytn")
        for t in range(ICO):
            nc.vector.tensor_mul(out=yt[:, t, :], in0=yp[:, t, :], in1=rb)
        for co in range(CO):
            op = psum_o.tile([P, LC], F32, tag="op")
            for t in range(ICO):
                nc.tensor.matmul(
                    out=op, lhsT=ww[:, t, P * co : P * (co + 1)], rhs=yt[:, t, :],
                    start=(t == 0), stop=(t == ICO - 1),
                )
            # An fp32 for the residual, recomputed on GPSIMD (ACT is busy
            # with exp; DVE with epilogues): relu(a*sc+sh) in two ops
            anr = work.tile([P, LC], F32, tag="anr")
            nc.gpsimd.tensor_scalar(
                out=anr, in0=araw[:, co, lsl],
                scalar1=scale["a"][:, co : co + 1], scalar2=shift["a"][:, co : co + 1],
                op0=mybir.AluOpType.mult, op1=mybir.AluOpType.add,
            )
            nc.gpsimd.tensor_scalar_max(out=anr, in0=anr, scalar1=0.0)
            ot = work.tile([P, LC], F32, tag="ot")
            nc.vector.scalar_tensor_tensor(
                out=ot, in0=op, scalar=wb[:, co : co + 1], in1=anr,
                op0=mybir.AluOpType.add, op1=mybir.AluOpType.add,
            )
            nc.sync.dma_start(out=out_dr[co][:, lsl], in_=ot)



def build_nc(n_cores=8):
    nc = bacc.Bacc("TRN2", target_bir_lowering=False, debug=False, num_devices=n_cores)

    def din(name, shape):
        dt = BF16 if name in WEIGHT_KEYS else F32
        return nc.dram_tensor(name, shape, dt, kind="ExternalInput").ap()

    ins = {
        "A": din("A", [C, L]),
        "B": din("B", [C, L]),
        "bn_gamma": din("bn_gamma", [C]),
        "bn_beta": din("bn_beta", [C]),
        "theta_w": din("theta_w", [C, IC]),
        "theta_b": din("theta_b", [IC]),
        "phi_w": din("phi_w", [C, IC]),
        "phi_b": din("phi_b", [IC]),
        "g_w": din("g_w", [C, IC]),
        "W_w": din("W_w", [IC, C]),
        "W_b": din("W_b", [C]),
    }
    outs = {"out": nc.dram_tensor("out", [C, L], F32, kind="ExternalOutput").ap()}
    with tile.TileContext(nc) as tc:
        emit_kernel(tc, outs, ins, n_cores)
    nc.compile()
    return nc


def fold_inputs(inputs):
    """g_b rides through the softmax unchanged (attn rows sum to 1), so it
    folds into the final bias: W_b_eff = W_b + g_b @ W_w (bf16 W_w to match
    the on-device projection)."""
    f = {k: np.asarray(v, dtype=np.float64) for k, v in inputs.items()}
    wwb = f["W_w"].astype(np.float32).astype(ml_dtypes.bfloat16).astype(np.float64)
    wb_eff = f["W_b"] + f["g_b"] @ wwb
    out = {k: v for k, v in inputs.items() if k != "g_b"}
    out["W_b"] = wb_eff.astype(np.float32)
    return out


def make_in_maps(inputs, n_cores):
    f = {}
    for k, v in fold_inputs(inputs).items():
        arr = np.ascontiguousarray(np.asarray(v), dtype=np.float32)
        if k in WEIGHT_KEYS:
            arr = arr.astype(ml_dtypes.bfloat16)
        f[k] = arr
    shared = {k: f[k] for k in f if k not in ("A", "B")}
    return [dict(A=f["A"][c], B=f["B"][c], **shared) for c in range(n_cores)]


_NC_CACHE = {}


def kernel(**inputs):
    n_cores = N
    if n_cores not in _NC_CACHE:
        _NC_CACHE[n_cores] = build_nc(n_cores)
    nc = _NC_CACHE[n_cores]
    in_maps = make_in_maps(inputs, n_cores)
    res = run_bass_kernel_spmd(nc, in_maps, core_ids=list(range(n_cores)))
    return np.stack([res.results[c]["out"] for c in range(n_cores)], axis=0)



# revision 29
# speedup vs baseline: 1.9189x; 1.9189x over previous
"""Trainium2 Bass kernel for nn_Cross_Attention_Network.

Reference computation (per batch element b of N=8, one per NeuronCore):
  An = relu(BN(A)) ; Bn = relu(BN(B))          # BN stats over (N, L) -> AllReduce
  q = An.T @ theta_w + theta_b                 # [L, IC]
  k = Bn.T @ phi_w + phi_b
  v = An.T @ g_w + g_b
  attn = softmax(q @ k.T, axis=-1)
  out = An.T + (attn @ v) @ W_w + W_b          # [L, C] -> emitted as [C, L]

Layout strategy (transpose-free):
  - A, B staged to bf16 host-side and live in native [C, L]; BN+ReLU is one
    fused ACT op per [128, 2048] tile producing An/Bn bf16 (An bf16 also
    serves as the residual source).
  - qT, kT computed in [IC, L] via bf16 matmuls; epilogues emit fp8e4m3.
  - scores computed directly transposed S^T[m, l] with fp8 DoubleRow matmuls
    (contraction ic=256 as 128 partitions x 2 packed -> 0.5 cycles/row).
  - softmax is max-free with a constant offset: exp(s - M_OFF) keeps the
    fp8e4m3 attention weights in range (scores <= ~8.5; e^{8.5-3} = 244 < 448);
    the offset cancels in the normalization.
  - exp runs on ACT over PAIRS of m-tiles ([128, 2, 512] PSUM reads across two
    banks) to halve instruction overhead; output pt is fp8.
  - v is fp8 with a literal ones-column appended; the y-stage DoubleRow
    matmuls produce y^T[ic, l] and the softmax denominator on the PE.
  - final projection W_w (fp8 DoubleRow) gives out^T[c, l]; bias on ACT,
    residual add (SBUF-only) on GPSIMD.
"""

from contextlib import ExitStack

import ml_dtypes
import numpy as np

try:
    import concourse.bass as bass  # noqa: F401
except ImportError:  # pragma: no cover - fallback for bare containers
    import sys

    sys.path.insert(0, "/opt/trn_rl_repo")
    import concourse.bass as bass  # noqa: F401

import concourse.tile as tile
from concourse import bacc, mybir
from concourse.bass_utils import run_bass_kernel_spmd
from concourse.masks import make_identity

N, C, L = 8, 512, 2048
IC = C // 2
P = 128
CO = C // P            # 4 channel tiles
ICO = IC // P          # 2 ic tiles
LC = 512               # l-chunk (PSUM bank width in fp32)
NLC = L // LC          # 4 l chunks
NM = L // P            # 16 m tiles
NP = NM // 2           # 8 m-tile pairs
BN_EPS = 1e-5
M_OFF = 3.0            # exp offset; cancels in normalization
F32 = mybir.dt.float32
BF16 = mybir.dt.bfloat16
F8 = mybir.dt.float8e4
DR = mybir.MatmulPerfMode.DoubleRow
BF16_KEYS = ("A", "B", "theta_w", "phi_w", "g_w")
F8_KEYS = ("W_w",)
DMA_SPLIT = 2
WORK_BUFS = 4


def dram_dtype(name):
    if name in BF16_KEYS:
        return BF16
    if name in F8_KEYS:
        return F8
    return F32


def declare_io(nc):
    def din(name, shape):
        return nc.dram_tensor(name, shape, dram_dtype(name),
                              kind="ExternalInput").ap()

    ins = {
        "A": din("A", [C, L]),
        "B": din("B", [C, L]),
        "bn_gamma": din("bn_gamma", [C]),
        "bn_beta": din("bn_beta", [C]),
        "theta_w": din("theta_w", [C, IC]),
        "theta_b": din("theta_b", [IC]),
        "phi_w": din("phi_w", [C, IC]),
        "phi_b": din("phi_b", [IC]),
        "g_w": din("g_w", [C, IC]),
        "W_w": din("W_w", [IC, C]),
        "W_b": din("W_b", [C]),
    }
    outs = {"out": nc.dram_tensor("out", [C, L], F32,
                                  kind="ExternalOutput").ap()}
    return ins, outs


def emit_kernel(tc, outs, ins, n_cores, use_cc=True, sections=("pre", "attn", "close"), state=None):
    if state is None:
        state = {}
    if "pre" in sections:
        _emit_pre(tc, outs, ins, n_cores, use_cc, state)
    if "attn" in sections:
        _emit_attn(tc, outs, ins, n_cores, state)
    if "close" in sections:
        emit_close(state)
    return state


def emit_close(state):
    state["ctx"].close()


def _emit_pre(tc, outs, ins, n_cores, use_cc, st):
    """Emit the per-core program. `outs`/`ins` are dicts of DRAM APs."""
    nc = tc.nc
    ctx = ExitStack()
    a_dr = ins["A"].rearrange("(o p) l -> o p l", p=P)
    b_dr = ins["B"].rearrange("(o p) l -> o p l", p=P)
    out_dr = outs["out"].rearrange("(o p) l -> o p l", p=P)

    const = ctx.enter_context(tc.tile_pool(name="const", bufs=1))
    big = ctx.enter_context(tc.tile_pool(name="big", bufs=1))
    stat = ctx.enter_context(tc.tile_pool(name="stat", bufs=1))
    dram = ctx.enter_context(tc.tile_pool(name="dram", bufs=1, space="DRAM"))
    work = ctx.enter_context(tc.tile_pool(name="work", bufs=WORK_BUFS))
    # PSUM budget (8 banks): sp pairs 2x2 + yp 2 + cs 1 + op 1
    psum_s = ctx.enter_context(tc.tile_pool(name="psum_s", bufs=2, space="PSUM"))
    psum_y = ctx.enter_context(tc.tile_pool(name="psum_y", bufs=1, space="PSUM"))
    psum_c = ctx.enter_context(tc.tile_pool(name="psum_c", bufs=1, space="PSUM"))
    psum_o = ctx.enter_context(tc.tile_pool(name="psum_o", bufs=1, space="PSUM"))
    raw_ctx = ExitStack()
    rawp = raw_ctx.enter_context(tc.tile_pool(name="raw", bufs=1))
    araw = rawp.tile([P, CO, L], BF16, tag="araw")
    braw = rawp.tile([P, CO, L], BF16, tag="braw")

    # ---- persistent SBUF arrays ----
    anb = big.tile([P, CO, L], BF16)      # An bf16 (matmul streams + residual)
    bnb = big.tile([P, CO, L], BF16)      # Bn bf16
    qT = big.tile([P, ICO, L], F8)        # q^T [IC, L] fp8
    kT = big.tile([P, ICO, L], F8)        # k^T [IC, L] fp8
    VROW = IC + 16        # 16B-aligned row stride for DoubleRow Ldweights
    vv = big.tile([P, NM, VROW], F8)      # v [L, IC | ones | pad] fp8
    thw = const.tile([P, CO, IC], BF16)
    phw = const.tile([P, CO, IC], BF16)
    gw = const.tile([P, CO, IC], BF16)
    ww = const.tile([P, ICO, C], F8)
    thb = const.tile([P, ICO], F32)
    phb = const.tile([P, ICO], F32)
    wb = const.tile([P, CO], F32)
    gam = const.tile([P, CO], F32)
    bet = const.tile([P, CO], F32)
    epst = const.tile([P, 1], F32)
    ident = const.tile([P, P], BF16)

    # ---- input DMAs: A then B (both stats chains gate the start), then
    # weights (first needed only at the qT matmuls) ----
    for o in range(CO):
        for h in range(DMA_SPLIT):
            hs = slice(h * (L // DMA_SPLIT), (h + 1) * (L // DMA_SPLIT))
            nc.sync.dma_start(out=araw[:, o, hs], in_=a_dr[o][:, hs])
    for o in range(CO):
        for h in range(DMA_SPLIT):
            hs = slice(h * (L // DMA_SPLIT), (h + 1) * (L // DMA_SPLIT))
            nc.sync.dma_start(out=braw[:, o, hs], in_=b_dr[o][:, hs])
    nc.sync.dma_start(out=gam, in_=ins["bn_gamma"].rearrange("(o p) -> p o", p=P))
    nc.sync.dma_start(out=bet, in_=ins["bn_beta"].rearrange("(o p) -> p o", p=P))
    nc.sync.dma_start(out=thw, in_=ins["theta_w"].rearrange("(o p) i -> p o i", p=P))
    nc.sync.dma_start(out=thb, in_=ins["theta_b"].rearrange("(o p) -> p o", p=P))
    nc.sync.dma_start(out=phw, in_=ins["phi_w"].rearrange("(o p) i -> p o i", p=P))
    nc.sync.dma_start(out=phb, in_=ins["phi_b"].rearrange("(o p) -> p o", p=P))
    nc.sync.dma_start(out=gw, in_=ins["g_w"].rearrange("(o p) i -> p o i", p=P))
    nc.sync.dma_start(out=ww, in_=ins["W_w"].rearrange("(o p) c -> p o c", p=P))
    nc.sync.dma_start(out=wb, in_=ins["W_b"].rearrange("(o p) -> p o", p=P))

    moff = const.tile([P, 1], F32)
    nc.vector.memset(epst, BN_EPS)
    nc.vector.memset(moff, -M_OFF)
    nc.vector.memset(vv[:, :, IC : IC + 1], 1.0)
    make_identity(nc, ident)
    wscr = stat.tile([P, LC], BF16, tag="warm_mm")
    wscr2 = stat.tile([P, LC], BF16, tag="warm_mm2")
    nc.vector.memset(wscr, 0.0)
    nc.vector.memset(wscr2, 0.0)
    for i in range(12):
        wpb = psum_s.tile([P, 2, LC], F32, tag="sp")
        nc.tensor.matmul(out=wpb[:, i % 2, :], lhsT=wscr[:, 0:P], rhs=wscr)
    # preload the Sqrt ACT table-set while input DMAs stream (the load is
    # ~2.7us and would otherwise land on the stats critical path)
    warm = stat.tile([P, 1], F32)
    nc.scalar.activation(out=warm, in_=epst, func=mybir.ActivationFunctionType.Sqrt)

    # ---- BN statistics: per-channel sum / sumsq over local L, AllReduce over cores
    replica = [list(range(n_cores))]
    scale = {}
    shift = {}

    ccgs = {}

    HL = L // DMA_SPLIT

    def emit_stats_collect(key, src_t, cc_engine, sumsq_on_act):
        # per-half accumulates so stats ops start as soon as each DMA chunk
        # lands; halves summed just before the AllReduce
        cc2 = stat.tile([P, CO, 2, 2], F32, tag=f"cc2_{key}")
        for o in range(CO):
            for h in range(DMA_SPLIT):
                hs = slice(HL * h, HL * (h + 1))
                scr = work.tile([P, HL], BF16, tag="stat_scr")
                nc.vector.tensor_scalar(
                    out=scr, in0=src_t[:, o, hs], scalar1=1.0, scalar2=0.0,
                    op0=mybir.AluOpType.mult, op1=mybir.AluOpType.add,
                    accum_out=cc2[:, o, h, 0:1],
                )
                if sumsq_on_act:
                    sq = work.tile([P, HL], BF16, tag="stat_sq")
                    nc.scalar.activation(
                        out=sq, in_=src_t[:, o, hs],
                        func=mybir.ActivationFunctionType.Square,
                        accum_out=cc2[:, o, h, 1:2],
                    )
                else:
                    sq = work.tile([P, HL], BF16, tag="stat_sq")
                    nc.vector.tensor_tensor(
                        out=sq, in0=src_t[:, o, hs], in1=src_t[:, o, hs],
                        op=mybir.AluOpType.mult,
                    )
                    scr2 = work.tile([P, HL], BF16, tag="stat_scr2")
                    nc.vector.tensor_scalar(
                        out=scr2, in0=sq, scalar1=1.0, scalar2=None,
                        op0=mybir.AluOpType.mult, accum_out=cc2[:, o, h, 1:2],
                    )
        ccin = stat.tile([P, CO, 2], F32, tag=f"ccin_{key}")
        nc.vector.tensor_tensor(out=ccin, in0=cc2[:, :, 0, :],
                                in1=cc2[:, :, 1, :], op=mybir.AluOpType.add)
        ccg = stat.tile([P, CO, 2], F32, tag=f"ccg_{key}")
        if n_cores > 1 and use_cc:
            cc_i = dram.tile([P, CO * 2], F32, tag=f"cci_{key}")
            cc_o = dram.tile([P, CO * 2], F32, tag=f"cco_{key}")
            nc.sync.dma_start(out=cc_i, in_=ccin)
            cc_engine.collective_compute(
                "AllReduce", mybir.AluOpType.add, replica_groups=replica,
                ins=[cc_i.opt()], outs=[cc_o.opt()],
            )
            nc.sync.dma_start(out=ccg, in_=cc_o)
        else:
            nc.vector.tensor_copy(out=ccg, in_=ccin)
        ccgs[key] = ccg

    def emit_stats_finish(key):
        ccg = ccgs[key]
        inv_cnt = 1.0 / float(n_cores * L)
        mean = stat.tile([P, CO], F32, tag=f"mean_{key}")
        msq = stat.tile([P, CO], F32, tag=f"msq_{key}")
        var = stat.tile([P, CO], F32, tag=f"var_{key}")
        sc = stat.tile([P, CO], F32, tag=f"sc_{key}")
        sh = stat.tile([P, CO], F32, tag=f"sh_{key}")
        nc.vector.tensor_scalar_mul(out=mean, in0=ccg[:, :, 0], scalar1=inv_cnt)
        nc.vector.tensor_scalar_mul(out=msq, in0=ccg[:, :, 1], scalar1=inv_cnt)
        nc.vector.tensor_mul(out=var, in0=mean, in1=mean)
        nc.vector.tensor_sub(out=var, in0=msq, in1=var)
        # std = sqrt(var + eps) on ACT, rstd via accurate DVE reciprocal
        nc.scalar.activation(out=var, in_=var, func=mybir.ActivationFunctionType.Sqrt, bias=epst)
        nc.vector.reciprocal(out=var, in_=var)
        nc.vector.tensor_mul(out=sc, in0=gam, in1=var)
        nc.vector.tensor_mul(out=sh, in0=mean, in1=sc)
        nc.vector.tensor_sub(out=sh, in0=bet, in1=sh)
        scale[key], shift[key] = sc, sh

    proj_idx = [0]
    proj_psb = [None]

    def project_T(dst, w, bias_pp, src_t, dve_chunks=()):
        # dst[ic, l] = sum_c w[c, ic] * src_t[c, l]; bias per-partition (ic).
        # chunk-major so the attention's first scores (which need both ic
        # tiles of chunk 0) unblock after two epilogues, not five.
        # Epilogues on ACT (Identity+bias, fp8 out) keep DVE free for relus;
        # chunks listed in dve_chunks drain on DVE instead (runs in parallel
        # with the ACT epilogue stream, unblocking the first scores earlier).
        for lc in range(NLC):
            for t in range(ICO):
                if proj_idx[0] % 2 == 0:
                    psb = psum_s.tile([P, 2, LC], F32, tag="sp")
                    proj_psb[0] = psb
                ps = proj_psb[0][:, proj_idx[0] % 2, :]
                proj_idx[0] += 1
                for o in range(CO):
                    nc.tensor.matmul(
                        out=ps, lhsT=w[:, o, P * t : P * (t + 1)],
                        rhs=src_t[:, o, LC * lc : LC * (lc + 1)],
                        start=(o == 0), stop=(o == CO - 1),
                    )
                dsl = dst[:, t, LC * lc : LC * (lc + 1)]
                if lc in dve_chunks:
                    nc.vector.tensor_scalar_add(out=dsl, in0=ps,
                                                scalar1=bias_pp[:, t : t + 1])
                else:
                    nc.scalar.activation(
                        out=dsl, in_=ps,
                        func=mybir.ActivationFunctionType.Identity,
                        bias=bias_pp[:, t : t + 1],
                    )

    def emit_relu_dve(dst, src, key):
        # BN affine + relu on DVE: 2 ops per tile, bf16 4x mode
        for o in range(CO):
            nc.vector.tensor_scalar(
                out=dst[:, o, :], in0=src[:, o, :],
                scalar1=scale[key][:, o : o + 1], scalar2=shift[key][:, o : o + 1],
                op0=mybir.AluOpType.mult, op1=mybir.AluOpType.add,
            )
            nc.vector.tensor_scalar_max(out=dst[:, o, :], in0=dst[:, o, :],
                                        scalar1=0.0)

    # Collect stats for BOTH sides and launch both AllReduces up front;
    # consume the results later, when each side's relu actually needs them.
    # sumsq on ACT for both sides (idle while the DMAs stream), sums on DVE.
    emit_stats_collect("a", araw, nc.gpsimd, sumsq_on_act=True)
    emit_stats_finish("a")
    emit_stats_collect("b", braw, nc.gpsimd, sumsq_on_act=True)
    # HAM re-warm bridge: gate junk matmuls on the AR-A result so the PE is
    # un-throttled when the first projection matmuls issue
    nc.vector.tensor_scalar_mul(out=wscr2[:, 0 : CO * 2],
                                in0=ccgs["a"].rearrange("p a b -> p (a b)"),
                                scalar1=0.0)
    for i in range(12):
        wpb = psum_s.tile([P, 2, LC], F32, tag="sp")
        nc.tensor.matmul(out=wpb[:, i % 2, :], lhsT=wscr2[:, 0:P], rhs=wscr2)
    nc.scalar.activation(out=warm, in_=epst, func=mybir.ActivationFunctionType.Exp)

    # relus on DVE (bf16 4x); An bf16 doubles as the residual source
    emit_relu_dve(anb, araw, "a")
    emit_stats_finish("b")
    emit_relu_dve(bnb, braw, "b")
    raw_ctx.close()

    project_T(qT, thw, thb, anb)
    project_T(kT, phw, phb, bnb, dve_chunks=(0,))

    st.update(dict(ctx=ctx, work=work, psum_s=psum_s, psum_y=psum_y,
                   psum_c=psum_c, psum_o=psum_o, anb=anb, qT=qT, kT=kT,
                   vv=vv, ww=ww, wb=wb, moff=moff, ident=ident, gw=gw,
                   scale=scale, shift=shift, out_dr=out_dr))


def _emit_attn(tc, outs, ins, n_cores, st):
    nc = tc.nc
    (work, psum_s, psum_y, psum_c, psum_o, anb, qT, kT, vv, ww, wb, moff,
     ident, gw, out_dr) = (
        st["work"], st["psum_s"], st["psum_y"], st["psum_c"], st["psum_o"],
        st["anb"], st["qT"], st["kT"], st["vv"], st["ww"], st["wb"],
        st["moff"], st["ident"], st["gw"], st["out_dr"])

    # ---- attention + output: flat software pipeline over (lc, mp) ----
    # scores run two pairs ahead of exp (psum_s bufs=2 suffices: scores for
    # pair n+2 fill the bank exp(n) just drained, while exp(n+1) runs); the
    # next chunk's score prologue is emitted before this chunk's output
    # stage so the in-order PE never parks exp behind the out-projection.
    def s_pair(lc, mp):
        # S^T[m, l]; fp8 DoubleRow packs the ic=256 contraction as 128 x 2
        lsl = slice(LC * lc, LC * (lc + 1))
        sp = psum_s.tile([P, 2, LC], F32, tag="sp")
        for i in range(2):
            m = 2 * mp + i
            nc.tensor.matmul(
                out=sp[:, i, :], lhsT=kT[:, 0:2, P * m : P * (m + 1)],
                rhs=qT[:, 0:2, lsl], perf_mode=DR,
            )
        return sp

    # Pipeline lags: scores for step n+2, exp for step n, y/cs for step n-2.
    # The per-chunk epilogue (recip/bcast/ynorm/out-proj) is sliced into
    # tasks spread across the NEXT chunk's iterations so every exp is
    # emitted before the epilogue work that would otherwise capture it in a
    # conservative cross-engine semaphore wait.
    order = [(lc, mp) for lc in range(NLC) for mp in range(NP)]
    NSTEP = len(order)
    sp_tiles = {}
    pt_tiles = {}
    yps = {}
    css = {}
    epi = {}  # lc -> dict of saved tiles for the spread-out epilogue

    def do_y(lc, mp):
        yp, cs = yps[lc], css[lc]
        pt = pt_tiles.pop((lc, mp))
        for t in range(ICO):
            nc.tensor.matmul(
                out=yp[:, t, :],
                lhsT=vv[:, 2 * mp : 2 * mp + 2, P * t : P * (t + 1)],
                rhs=pt, perf_mode=DR,
                start=(mp == 0), stop=(mp == NP - 1), skip_group_check=True,
            )
        nc.tensor.matmul(
            out=cs, lhsT=vv[:, 2 * mp : 2 * mp + 2, IC : IC + 1],
            rhs=pt, perf_mode=DR,
            start=(mp == 0), stop=(mp == NP - 1), skip_group_check=True,
        )

    def do_epi_task(lc, k):
        e = epi[lc]
        if k == 0:
            rec = work.tile([1, LC], F32, tag="rec")
            nc.vector.reciprocal_approx_fast(out=rec, in_=css[lc])
            rb = work.tile([P, LC], F32, tag="rb")
            nc.gpsimd.partition_broadcast(out_ap=rb, in_ap=rec)
            e["rb"] = rb  # noqa
        elif k == 1:
            yt = work.tile([P, ICO, LC], F8, tag="ytn")
            for t in range(ICO):
                nc.vector.tensor_mul(out=yt[:, t, :], in0=yps[lc][:, t, :],
                                     in1=e["rb"])
            e["yt"] = yt
        else:
            co = k - 2
            lsl = slice(LC * lc, LC * (lc + 1))
            op = psum_o.tile([P, LC], F32, tag="op")
            nc.tensor.matmul(
                out=op, lhsT=ww[:, 0:2, P * co : P * (co + 1)],
                rhs=e["yt"][:, 0:2, :], perf_mode=DR, start=True, stop=False,
            )
            # residual accumulated on the PE: op += I @ An[c-tile, lsl]
            nc.tensor.matmul(
                out=op, lhsT=ident, rhs=anb[:, co, lsl],
                start=False, stop=True,
            )
            ot = work.tile([P, LC], F32, tag="ot")
            nc.vector.tensor_scalar_add(out=ot, in0=op,
                                        scalar1=wb[:, co : co + 1])
            nc.sync.dma_start(out=out_dr[co][:, lsl], in_=ot)
            if co == CO - 1:
                del yps[lc], css[lc], epi[lc]

    def do_v_pair(j):
        # v[m, ic] = sum_c An[c, m] g_w[c, ic]; interleaved into the first
        # chunk's iterations so the PE never parks attention behind it
        # (g_b folded into W_b host-side); copies on DVE (ACT runs exp)
        for i in range(2):
            m = 2 * j + i
            ps = psum_o.tile([P, LC], F32, tag="op")
            for o in range(CO):
                nc.tensor.matmul(
                    out=ps[:, 0:IC], lhsT=anb[:, o, P * m : P * (m + 1)],
                    rhs=gw[:, o, :], start=(o == 0), stop=(o == CO - 1),
                )
            nc.vector.tensor_copy(out=vv[:, m, 0:IC], in_=ps[:, 0:IC])

    sp_tiles[order[0]] = s_pair(*order[0])
    sp_tiles[order[1]] = s_pair(*order[1])
    for n, (lc, mp) in enumerate(order):
        if mp == 0:
            yp_t = psum_y.tile([P, 2, LC], F32, tag="yp")
            cs_t = psum_c.tile([1, LC], F32, tag="cs")
            yps[lc], css[lc], epi[lc] = yp_t, cs_t, {}
        pt = work.tile([P, 2, LC], F8, tag="pt")
        nc.scalar.activation(out=pt, in_=sp_tiles.pop((lc, mp)),
                             func=mybir.ActivationFunctionType.Exp,
                             bias=moff)
        pt_tiles[(lc, mp)] = pt
        if n + 2 < NSTEP:
            sp_tiles[order[n + 2]] = s_pair(*order[n + 2])
        if lc == 0:
            do_v_pair(mp)
        if lc > 0:
            # k=0 (recip) and k=1 (ynorm) both at mp=2: ynorm's read of the
            # retired yp bank must precede do_y((lc,0))'s reuse of it below
            for k in {2: (0, 1), 3: (2,), 4: (3,), 5: (4,), 6: (5,)}.get(mp, ()):
                do_epi_task(lc - 1, k)
        if n >= 2:
            do_y(*order[n - 2])
    # drain: last two y/cs steps, then the final chunk's epilogue
    do_y(*order[NSTEP - 2])
    do_y(*order[NSTEP - 1])
    for k in range(2 + CO):
        do_epi_task(NLC - 1, k)


def build_nc(n_cores=8):
    nc = bacc.Bacc("TRN2", target_bir_lowering=False, debug=False, num_devices=n_cores)
    ins, outs = declare_io(nc)
    with tile.TileContext(nc) as tc:
        emit_kernel(tc, outs, ins, n_cores)
    nc.compile()
    return nc


def fold_inputs(inputs):
    """g_b rides through the softmax unchanged (attn rows sum to 1), so it
    folds into the final bias: W_b_eff = W_b + g_b @ W_w (fp8-quantized W_w
    to match the on-device projection)."""
    f = {k: np.asarray(v, dtype=np.float64) for k, v in inputs.items()}
    wwq = f["W_w"].astype(np.float32).astype(mybir.dt.np(F8)).astype(np.float64)
    wb_eff = f["W_b"] + f["g_b"] @ wwq
    out = {k: v for k, v in inputs.items() if k != "g_b"}
    out["W_b"] = wb_eff.astype(np.float32)
    return out


def make_in_maps(inputs, n_cores):
    f = {}
    for k, v in fold_inputs(inputs).items():
        arr = np.ascontiguousarray(np.asarray(v), dtype=np.float32)
        dt = dram_dtype(k)
        if dt == BF16:
            arr = arr.astype(ml_dtypes.bfloat16)
        elif dt == F8:
            arr = arr.astype(mybir.dt.np(F8))
        f[k] = arr
    shared = {k: f[k] for k in f if k not in ("A", "B")}
    return [dict(A=f["A"][c], B=f["B"][c], **shared) for c in range(n_cores)]


_NC_CACHE = {}


def kernel(**inputs):
    n_cores = N
    if n_cores not in _NC_CACHE:
        _NC_CACHE[n_cores] = build_nc(n_cores)
    nc = _NC_CACHE[n_cores]
    in_maps = make_in_maps(inputs, n_cores)
    res = run_bass_kernel_spmd(nc, in_maps, core_ids=list(range(n_cores)))
    return np.stack([res.results[c]["out"] for c in range(n_cores)], axis=0)
